# revision 12
# baseline (speedup 1.0000x reference)
"""GATv2 GNN (4 layers + head) on 8 trn2 NeuronCores via Bass/Tile.

Sharding: nodes partitioned 1000/core (padded to 1024 rows); edges assigned to
the core owning their destination; weights SHARDED across cores (1/8 each) and
AllGathered on device (host->device transfer is the bottleneck under axon, the
device interconnect is comparatively free). Per layer:
  - feature matmuls xla = h @ (Wl * a), xr_a = h @ (Wr * a)   [a folded into W]
  - AllGather of xla shards (bf16) -> per-core DRAM copy of all source rows
  - dma_gather of source/dest rows per edge slot (128 edges per slot)
  - attention scores via sign-split leaky-relu accumulation on ScalarE:
      e = sum_c a_c*LR(u_c) = sum_{a>0} LR(w) - sum_{a<0} LR(-w),  w = a*u
  - softmax without max-shift (exp directly; segment denominators via the
    same one-hot S0 matmuls that aggregate the numerator)
  - numer[d,:] = sum_e S0[d,e]*ex_e*xls_e on TensorE, per 128-dst block
  - BN (train-mode) with cross-core AllReduce of sum/sumsq; BN absorbs the
    a-scaling exactly via sign-folded gamma and per-channel eps*a^2.

Transfer-mins: S0 one-hots and the transpose identity are generated on device
(iota + is_equal); gather indices are sent compact ([16, n/16], replicated to
128 partitions on device); h0T is sent unpadded [3201, 1000]; all small
per-channel vectors ride in one packed f32 tensor.
"""

import os
import sys
from contextlib import ExitStack

import numpy as np
import ml_dtypes

sys.path.insert(0, "/opt/trn_rl_repo")

import concourse.bass as bass  # noqa: E402
import concourse.tile as tile  # noqa: E402
from concourse import bacc, mybir  # noqa: E402
from concourse import bass2jax as _b2j  # noqa: E402
from concourse.bass_utils import run_bass_kernel_spmd  # noqa: E402

# ---------------------------------------------------------------------------
# run_bass_kernel_spmd (under axon) rebuilds and re-traces a fresh jax.jit on
# every call (~0.4-0.5s of pure client-side overhead). Cache the traced
# callable per (Bass module, n_cores) -- semantics are identical (inputs are
# still shipped and the NEFF still executes on every call).
_PJRT_CACHE = {}
_ORIG_RUN_VIA_PJRT = _b2j.run_bass_via_pjrt


def _cached_run_bass_via_pjrt(nc, in_maps, n_cores):
    import jax
    from jax.sharding import Mesh, PartitionSpec
    from jax.experimental.shard_map import shard_map

    if nc.dbg_addr is not None or n_cores <= 1:
        return _ORIG_RUN_VIA_PJRT(nc, in_maps, n_cores)
    key = (id(nc), n_cores)
    ent = _PJRT_CACHE.get(key)
    if ent is None:
        _b2j.install_neuronx_cc_hook()
        partition_name = (nc.partition_id_tensor.name
                          if nc.partition_id_tensor else None)
        in_names, out_names, out_avals = [], [], []
        for alloc in nc.m.functions[0].allocations:
            if not isinstance(alloc, mybir.MemoryLocationSet):
                continue
            name = alloc.memorylocations[0].name
            if alloc.kind == "ExternalInput":
                if name != partition_name:
                    in_names.append(name)
            elif alloc.kind == "ExternalOutput":
                out_names.append(name)
                out_avals.append(jax.core.ShapedArray(
                    tuple(alloc.tensor_shape), mybir.dt.np(alloc.dtype)))
        n_params = len(in_names)
        in_names_full = (in_names + out_names
                         + ([partition_name] if partition_name else []))
        donate = tuple(range(n_params, n_params + len(out_names)))

        def _body(*args):
            operands = list(args)
            if partition_name is not None:
                operands.append(_b2j.partition_id_tensor())
            return tuple(_b2j._bass_exec_p.bind(
                *operands, out_avals=tuple(out_avals),
                in_names=tuple(in_names_full), out_names=tuple(out_names),
                lowering_input_output_aliases=(), sim_require_finite=True,
                sim_require_nnan=True, nc=nc))

        devices = jax.devices()[:n_cores]
        assert len(devices) == n_cores
        mesh = Mesh(np.asarray(devices), ("core",))
        sharded = jax.jit(
            shard_map(_body, mesh=mesh,
                      in_specs=(PartitionSpec("core"),)
                      * (n_params + len(out_names)),
                      out_specs=(PartitionSpec("core"),) * len(out_names),
                      check_rep=False),
            donate_argnums=donate, keep_unused=True)
        # donated (zero-init) output buffers are created on-device: cheaper
        # than shipping zero arrays from the host
        import jax.numpy as jnp
        from jax.sharding import NamedSharding

        sh_ = NamedSharding(mesh, PartitionSpec("core"))
        zeros_fns = [
            jax.jit(lambda a=a: jnp.zeros((n_cores * a.shape[0],
                                           *a.shape[1:]), a.dtype),
                    out_shardings=sh_)
            for a in out_avals]
        ent = (sharded, in_names, out_names, out_avals, zeros_fns)
        _PJRT_CACHE[key] = ent
    sharded, in_names, out_names, out_avals, zeros_fns = ent
    concat_in = [np.concatenate([np.asarray(m[name]) for m in in_maps], axis=0)
                 for name in in_names]
    concat_zeros = [f() for f in zeros_fns]
    out_arrs = sharded(*concat_in, *concat_zeros)
    return [
        {name: np.asarray(out_arrs[i]).reshape(n_cores, *out_avals[i].shape)[c]
         for i, name in enumerate(out_names)}
        for c in range(n_cores)
    ]


_b2j.run_bass_via_pjrt = _cached_run_bass_via_pjrt
# ---------------------------------------------------------------------------

NC = 8
N = 8000
NPC = 1000
ROWS = 1024
F_IN = 3201
F_PAD = 3328  # 26 * 128
GS = 4        # slots per dma_gather group (all layers)
BF = ml_dtypes.bfloat16

# (Cin_pad, Cout, H, Cc)
LAYERS = [(F_PAD, 1024, 2, 512), (1024, 512, 1, 512),
          (512, 512, 1, 512), (512, 512, 1, 512)]

# row layout of the merged f32 weight-shard tensor (per core: [512, 512];
# AllGathered to [4096, 512] with rank r's chunk at rows r*512)
WFS_OFF = {1: (0, 128), 2: (256, 64), 3: (384, 64)}  # li -> (row0_l, rows)


def _groups(slots, gs):
    g, s = [], 0
    while s < slots:
        g.append((s, min(gs, slots - s)))
        s += min(gs, slots - s)
    return g


def _wrap_idx(idx_flat, slots):
    """Pack a flat idx list into compact [16, n/16] column-major-16 wrapped
    layout, independently per dma_gather group (GS slots each). The device
    replicates to 128 partitions."""
    cols = []
    for g0, gs in _groups(slots, GS):
        part = idx_flat[g0 * 128:(g0 + gs) * 128]
        cols.append(np.ascontiguousarray(part.reshape(-1, 16).T))
    return np.concatenate(cols, axis=1).astype(np.int16)


def build_structs(edge_index):
    src = np.concatenate([edge_index[0], np.arange(N)]).astype(np.int64)
    dst = np.concatenate([edge_index[1], np.arange(N)]).astype(np.int64)
    deg = np.bincount(dst, minlength=N).astype(np.float32)

    core_of = dst // NPC
    dst_local = dst % NPC
    blk = dst_local // 128
    lists = [[np.nonzero((core_of == c) & (blk == b))[0] for b in range(8)]
             for c in range(NC)]
    S = [max(int(np.ceil(len(lists[c][b]) / 128)) for c in range(NC))
         for b in range(8)]
    off = np.concatenate([[0], np.cumsum(S)]).astype(int)
    SLOTS = int(off[-1])

    src_pos = np.zeros((NC, SLOTS * 128), np.int16)
    dst_pos = np.zeros((NC, SLOTS * 128), np.int16)
    # dst_adj[c, p, s] = dst_local - blk*128 for the edge at lane p of slot s
    # (i.e. the one-hot column), or a sentinel for padding lanes.
    dst_adj = np.full((NC, 128, SLOTS), 9999.0, np.float32)
    for c in range(NC):
        for b in range(8):
            e = lists[c][b]
            e = e[np.lexsort((src[e], dst[e]))]
            L = off[b] * 128 + np.arange(len(e))
            src_pos[c, L] = ((src[e] // NPC) * ROWS + (src[e] % NPC)).astype(np.int16)
            dst_pos[c, L] = dst_local[e].astype(np.int16)
            dst_adj[c, L % 128, L // 128] = (dst_local[e] - b * 128).astype(np.float32)
    blk_of_slot = np.concatenate([[b] * S[b] for b in range(8)]).astype(int)
    return dict(deg=deg, S=S, off=off, SLOTS=SLOTS, src_pos=src_pos,
                dst_pos=dst_pos, dst_adj=dst_adj, blk_of_slot=blk_of_slot)


def prep_weights(inputs):
    """Sign-sort channels per head, fold a into W columns, fold sign(a) and
    eps*a^2 into BN; permute consumer rows."""
    out = {}
    prev_perm = None
    npos_all = []
    for li, (cin, cout, H, Cc) in enumerate(LAYERS):
        wl = np.asarray(inputs[f"W{li + 1}l"]).astype(np.float64)
        wr = np.asarray(inputs[f"W{li + 1}r"]).astype(np.float64)
        a = np.asarray(inputs[f"a{li + 1}"]).reshape(H, Cc).astype(np.float64)
        if prev_perm is not None:
            wl = wl[prev_perm]
            wr = wr[prev_perm]
        perm = np.zeros(H * Cc, int)
        npos = []
        for h in range(H):
            ph = np.argsort(~(a[h] > 0), kind="stable")
            perm[h * Cc:(h + 1) * Cc] = h * Cc + ph
            npos.append(int((a[h] > 0).sum()))
        npos_all.append(npos)
        a_s = a.reshape(-1)[perm]
        wl = wl[:, perm] * a_s[None, :]
        wr = wr[:, perm] * a_s[None, :]
        out[f"wl{li}"] = wl.astype(np.float32)
        out[f"wr{li}"] = wr.astype(np.float32)
        if li < 3:
            g = np.asarray(inputs[f"bn{li + 1}_g"])[perm] * np.sign(a_s)
            b = np.asarray(inputs[f"bn{li + 1}_b"])[perm]
            eps = 1e-5 * a_s * a_s
            out[f"bn{li}"] = (g.astype(np.float32), b.astype(np.float32),
                              eps.astype(np.float32))
        else:
            out["scale4"] = (1.0 / a_s).astype(np.float32)
            out["bias4"] = np.asarray(inputs["b4"])[perm].astype(np.float32)
        prev_perm = perm
    out["wh"] = np.asarray(inputs["Wh"])[prev_perm].astype(np.float32)
    out["npos"] = npos_all
    return out


def _pack_pp(vec):
    """[k*128] -> [128, k] per-partition packing (chunk c in column c)."""
    k = len(vec) // 128
    return np.ascontiguousarray(vec.reshape(k, 128).T).astype(np.float32)


_PROGRAM_CACHE = {}


KSTAGES = int(os.environ.get("KSTAGES", "99"))


def build_program(G, npos, bh_val):
    key = (tuple(G["S"]), tuple(tuple(x) for x in npos), float(bh_val), KSTAGES)
    if key in _PROGRAM_CACHE:
        return _PROGRAM_CACHE[key]

    SLOTS = G["SLOTS"]
    off = G["off"]
    blk_of_slot = G["blk_of_slot"]
    f32, f32r, bf16, i16, i32 = (mybir.dt.float32, mybir.dt.float32r,
                                 mybir.dt.bfloat16, mybir.dt.int16,
                                 mybir.dt.int32)
    AF = mybir.ActivationFunctionType
    ALU = mybir.AluOpType

    nc = bacc.Bacc("TRN2", target_bir_lowering=False, debug=False,
                   num_devices=NC)

    # ---------------- inputs (transfer-minimized: 2 packed blobs)
    # blob16 (bf16): [h0p (3201*1000) | w0l shard | w0r shard | isrc | idst]
    # blob32 (f32):  [wfs shard (512*512) | cst (128*CST_COLS)]
    CST_COLS = SLOTS + 8 + 8 + 8 + 4 + 4 + (8 * 3) + (4 * 3) + (4 * 3)
    SH = F_PAD // NC
    NIDX = 16 * SLOTS * 8
    OW0L = F_IN * NPC
    OW0R = OW0L + SH * 1024
    OIS = OW0R + SH * 1024
    OID = OIS + NIDX
    NB16 = OID + NIDX
    OCST = 512 * 512
    NB32 = OCST + 128 * CST_COLS
    b16_d = nc.dram_tensor("b16", [1, NB16], bf16, kind="ExternalInput")
    b32_d = nc.dram_tensor("b32", [1, NB32], f32, kind="ExternalInput")
    pred_d = nc.dram_tensor("pred", [1, ROWS], f32, kind="ExternalOutput")

    def bview(dram_t, a, n, w):
        """[1, NB] blob -> [n/w, w] AP starting at element a."""
        return dram_t[:][0:1, a:a + n].rearrange("o (p w) -> (o p) w", w=w)

    co = {}
    _c = SLOTS
    for nm, w in (("whp", 8), ("invdeg", 8), ("dmy", 8), ("sc4", 4),
                  ("b4p", 4), ("bn00", 8), ("bn01", 8), ("bn02", 8),
                  ("bn10", 4), ("bn11", 4), ("bn12", 4), ("bn20", 4),
                  ("bn21", 4), ("bn22", 4)):
        co[nm] = (_c, _c + w)
        _c += w
    assert _c == CST_COLS

    with tile.TileContext(nc) as tc, ExitStack() as top:
        dram = top.enter_context(tc.tile_pool(name="dram", bufs=1, space="DRAM"))
        const_p = top.enter_context(tc.tile_pool(name="const", bufs=1))
        s0_p = top.enter_context(tc.tile_pool(name="s0p", bufs=1))

        # -------- gathered weights in DRAM (Shared for fast AllGather)
        w0l_full = dram.tile([F_PAD, 1024], bf16, tag="w0lf", name="w0lf",
                             addr_space="Shared")
        w0r_full = dram.tile([F_PAD, 1024], bf16, tag="w0rf", name="w0rf",
                             addr_space="Shared")
        wfs_full = dram.tile([NC * 512, 512], f32, tag="wfsf", name="wfsf",
                             addr_space="Shared")
        # collectives cannot read IO tensors -> stage shards in Internal DRAM
        for nm, view, shp, dt_, outd in (
                ("w0l", bview(b16_d, OW0L, SH * 1024, 1024), [SH, 1024], bf16,
                 w0l_full),
                ("w0r", bview(b16_d, OW0R, SH * 1024, 1024), [SH, 1024], bf16,
                 w0r_full),
                ("wfs", bview(b32_d, 0, 512 * 512, 512), [512, 512], f32,
                 wfs_full)):
            stg = dram.tile(shp, dt_, tag=f"stg_{nm}", name=f"stg_{nm}")
            nc.sync.dma_start(stg[:], view)
            nc.gpsimd.collective_compute(
                "AllGather", ALU.bypass,
                replica_groups=[list(range(NC))],
                ins=[stg[:].opt()], outs=[outd[:].opt()])

        # -------- small constants
        cst = const_p.tile([128, CST_COLS], f32, tag="cst", name="cst")
        nc.sync.dma_start(cst[:], bview(b32_d, OCST, 128 * CST_COLS, CST_COLS))

        def cv(nm):
            a, b = co[nm]
            return cst[:, a:b]

        invdeg, dummy = cv("invdeg"), cv("dmy")
        sc4, b4p = cv("sc4"), cv("b4p")
        bnv = {0: (cv("bn00"), cv("bn01"), cv("bn02")),
               1: (cv("bn10"), cv("bn11"), cv("bn12")),
               2: (cv("bn20"), cv("bn21"), cv("bn22"))}

        # -------- gather indices: replicate [16, n] -> [128, n]
        isrc = const_p.tile([128, SLOTS * 8], i16, tag="isrc", name="isrc")
        idst = const_p.tile([128, SLOTS * 8], i16, tag="idst", name="idst")
        is_v = bview(b16_d, OIS, NIDX, SLOTS * 8).bitcast(i16)
        id_v = bview(b16_d, OID, NIDX, SLOTS * 8).bitcast(i16)
        for r in range(8):
            nc.sync.dma_start(isrc[r * 16:(r + 1) * 16, :], is_v)
            nc.sync.dma_start(idst[r * 16:(r + 1) * 16, :], id_v)

        # -------- iota / eye / S0 one-hots generated on device
        # (values <= 128 are exact in f32, so direct f32 iota is safe)
        iotaJ = const_p.tile([128, 128], f32, tag="iotaJ", name="iotaJ")
        nc.gpsimd.iota(iotaJ[:], pattern=[[1, 128]], base=0,
                       channel_multiplier=0,
                       allow_small_or_imprecise_dtypes=True)
        iotaP = const_p.tile([128, 1], f32, tag="iotaP", name="iotaP")
        nc.gpsimd.iota(iotaP[:], pattern=[[0, 1]], base=0,
                       channel_multiplier=1,
                       allow_small_or_imprecise_dtypes=True)
        eye = const_p.tile([128, 128], f32, tag="eye", name="eye")
        nc.vector.tensor_scalar(eye[:], iotaJ[:], iotaP[:, 0:1], None,
                                op0=ALU.is_equal)
        s0_sb = s0_p.tile([128, SLOTS * 128], bf16)
        for s in range(SLOTS):
            nc.vector.tensor_scalar(s0_sb[:, s * 128:(s + 1) * 128], iotaJ[:],
                                    cst[:, s:s + 1], None, op0=ALU.is_equal)

        xla_sh, xla_full, xr_loc = {}, {}, {}
        for li, (_, cout, _, _) in enumerate(LAYERS):
            xla_sh[li] = dram.tile([ROWS, cout], bf16, tag=f"xlash{li}", name=f"xlash{li}")
            xla_full[li] = dram.tile([NC * ROWS, cout], bf16, tag=f"xlaf{li}",
                                     name=f"xlaf{li}", addr_space="Shared")
            xr_loc[li] = dram.tile([ROWS, cout], bf16, tag=f"xrloc{li}", name=f"xrloc{li}")

        # hT pools managed non-nested (layer li's hT dies after its F phase)
        hT_pool = {0: tc.alloc_tile_pool(name="hT0", bufs=1)}
        hT = []
        for k in range(F_PAD // 128):
            t = hT_pool[0].tile([128, ROWS], bf16, tag=f"h{k}", name=f"hT0_{k}")
            if k < 25:
                nc.gpsimd.memset(t[:, NPC:ROWS], 0.0)
                nc.sync.dma_start(t[:, 0:NPC],
                                  bview(b16_d, k * 128 * NPC, 128 * NPC, NPC))
            else:
                nc.gpsimd.memset(t[:], 0.0)
                nc.sync.dma_start(t[0:1, 0:NPC],
                                  bview(b16_d, 3200 * NPC, NPC, NPC))
            hT.append(t)

        for li, (cin, cout, H, Cc) in enumerate(LAYERS):
            kc = cin // 128
            nch_out = cout // 128
            mm_dt = bf16 if li == 0 else f32r
            if 4 * li + 0 >= KSTAGES:
                break

            # ================= feature phase =================
            with ExitStack() as lf:
                fpsum = lf.enter_context(
                    tc.tile_pool(name=f"fps{li}", bufs=1 if li == 0 else 2,
                                 space="PSUM"))
                fout = lf.enter_context(tc.tile_pool(name=f"fo{li}", bufs=4))
                wpool = lf.enter_context(tc.tile_pool(name=f"w{li}", bufs=1))
                wsp = lf.enter_context(tc.tile_pool(name=f"ws{li}", bufs=10))

                if li == 0:
                    # W streamed: for each n-half and m-group of 4, stream K
                    for nh in range(2):
                        nsl = slice(nh * 512, (nh + 1) * 512)
                        for mg in range(2):
                            psl = [fpsum.tile([128, 512], f32, tag=f"psl{j}", name=f"psl{j}") for j in range(4)]
                            psr = [fpsum.tile([128, 512], f32, tag=f"psr{j}", name=f"psr{j}") for j in range(4)]
                            for k in range(kc):
                                tl = wsp.tile([128, 512], bf16, tag="wls")
                                nc.sync.dma_start(
                                    tl[:], w0l_full[:].rearrange(
                                        "(k p) n -> k p n", p=128)[k, :, nsl])
                                tr = wsp.tile([128, 512], bf16, tag="wrs")
                                nc.sync.dma_start(
                                    tr[:], w0r_full[:].rearrange(
                                        "(k p) n -> k p n", p=128)[k, :, nsl])
                                st, sp0 = k == 0, k == kc - 1
                                for j in range(4):
                                    m = mg * 4 + j
                                    lhsT = hT[k][:, m * 128:(m + 1) * 128]
                                    nc.tensor.matmul(psl[j][:], lhsT, tl[:],
                                                     start=st, stop=sp0)
                                    nc.tensor.matmul(psr[j][:], lhsT, tr[:],
                                                     start=st, stop=sp0)
                            for j in range(4):
                                m = mg * 4 + j
                                rsl = slice(m * 128, (m + 1) * 128)
                                xla_m = fout.tile([128, 512], bf16, tag="xlam")
                                nc.scalar.activation(xla_m[:], psl[j][:], AF.Copy)
                                nc.sync.dma_start(xla_sh[li][rsl, nsl], xla_m[:])
                                xr_m = fout.tile([128, 512], bf16, tag="xrm")
                                nc.scalar.activation(xr_m[:], psr[j][:], AF.Copy)
                                nc.sync.dma_start(xr_loc[li][rsl, nsl], xr_m[:])
                else:
                    wfs_v = wfs_full[:].rearrange("(c r) n -> c r n", r=512)
                    r0, rw = WFS_OFF[li]
                    wl_t, wr_t = [], []
                    for k in range(kc):
                        tl = wpool.tile([128, cout], mm_dt, tag=f"wl{k}")
                        tr = wpool.tile([128, cout], mm_dt, tag=f"wr{k}")
                        if rw == 128:
                            nc.gpsimd.dma_start(tl[:], wfs_v[k][r0:r0 + 128, :])
                            nc.gpsimd.dma_start(tr[:], wfs_v[k][r0 + 128:r0 + 256, :])
                        else:
                            # k-tile spans two rank chunks of 64 rows each
                            for h2 in range(2):
                                nc.gpsimd.dma_start(
                                    tl[h2 * 64:(h2 + 1) * 64, :],
                                    wfs_v[2 * k + h2][r0:r0 + 64, :])
                                nc.gpsimd.dma_start(
                                    tr[h2 * 64:(h2 + 1) * 64, :],
                                    wfs_v[2 * k + h2][r0 + 64:r0 + 128, :])
                        wl_t.append(tl)
                        wr_t.append(tr)
                    for m in range(8):
                        psl = fpsum.tile([128, cout], f32, tag="psl")
                        psr = fpsum.tile([128, cout], f32, tag="psr")
                        for k in range(kc):
                            lhsT = hT[k][:, m * 128:(m + 1) * 128]
                            st, sp0 = k == 0, k == kc - 1
                            nc.tensor.matmul(psl[:], lhsT, wl_t[k][:],
                                             start=st, stop=sp0)
                            nc.tensor.matmul(psr[:], lhsT, wr_t[k][:],
                                             start=st, stop=sp0)
                        rsl = slice(m * 128, (m + 1) * 128)
                        xla_m = fout.tile([128, cout], bf16, tag="xlam")
                        nc.scalar.activation(xla_m[:], psl[:], AF.Copy)
                        nc.sync.dma_start(xla_sh[li][rsl, :], xla_m[:])
                        xr_m = fout.tile([128, cout], bf16, tag="xrm")
                        nc.scalar.activation(xr_m[:], psr[:], AF.Copy)
                        nc.sync.dma_start(xr_loc[li][rsl, :], xr_m[:])

            hT_pool[li].release()  # free this layer's hT
            nch_out_ = cout // 128
            hT_pool[li + 1] = tc.alloc_tile_pool(name=f"hT{li + 1}", bufs=1)
            hT_next = [hT_pool[li + 1].tile([128, ROWS], f32r, tag=f"h{c}",
                                            name=f"hT{li + 1}_{c}")
                       for c in range(nch_out_)]

            if 4 * li + 1 >= KSTAGES:
                break
            nc.gpsimd.collective_compute(
                "AllGather", ALU.bypass,
                replica_groups=[list(range(NC))],
                ins=[xla_sh[li][:].opt()],
                outs=[xla_full[li][:].opt()],
            )
            if 4 * li + 2 >= KSTAGES:
                break

            # ================= edge phase =================
            aggp = tc.alloc_tile_pool(name=f"agg{li}", bufs=1)
            agg_full = aggp.tile([128, 8, cout], f32, tag="agg")
            with ExitStack() as le:
                gp = le.enter_context(tc.tile_pool(name=f"g{li}", bufs=3))
                wp = le.enter_context(tc.tile_pool(name=f"wt{li}", bufs=2))
                sp_ = le.enter_context(tc.tile_pool(name=f"sm{li}", bufs=4))
                scp = le.enter_context(tc.tile_pool(name=f"scr{li}", bufs=8))
                epsum = le.enter_context(
                    tc.tile_pool(name=f"eps{li}", bufs=2, space="PSUM"))

                numer_ps = denom_ps = None
                for g0, gs in _groups(SLOTS, GS):
                    xls = gp.tile([128, GS, cout], bf16, tag="xls")
                    nc.gpsimd.dma_gather(
                        xls[:, 0:gs, :], xla_full[li][:],
                        isrc[:, g0 * 8:(g0 + gs) * 8], gs * 128, gs * 128, cout)
                    xrg = gp.tile([128, GS, cout], bf16, tag="xrg")
                    nc.gpsimd.dma_gather(
                        xrg[:, 0:gs, :], xr_loc[li][:],
                        idst[:, g0 * 8:(g0 + gs) * 8], gs * 128, gs * 128, cout)
                    wt = wp.tile([128, GS, cout], bf16, tag="wt")
                    nc.vector.tensor_add(wt[:, 0:gs, :], xls[:, 0:gs, :],
                                         xrg[:, 0:gs, :])
                    pq = sp_.tile([128, GS, H, 2], f32, tag="pq")
                    for si in range(gs):
                        for h in range(H):
                            b0 = h * Cc
                            nph = npos[li][h]
                            # evaluate LR at 16x scale (LUT abs-error there
                            # is cheaper); 1/16 folded into the Exp scale
                            scr = scp.tile([128, 512], bf16, tag="scr")
                            nc.scalar.activation(
                                scr[:, 0:nph], wt[:, si, b0:b0 + nph],
                                AF.Prelu, scale=16.0, alpha=0.2,
                                accum_out=pq[:, si, h, 0:1])
                            scr2 = scp.tile([128, 512], bf16, tag="scr")
                            nc.scalar.activation(
                                scr2[:, 0:Cc - nph], wt[:, si, b0 + nph:b0 + Cc],
                                AF.Prelu, scale=-16.0, alpha=0.2,
                                accum_out=pq[:, si, h, 1:2])
                    esc = sp_.tile([128, GS, H], f32, tag="esc")
                    nc.vector.tensor_tensor(
                        esc[:, 0:gs, :], pq[:, 0:gs, :, 0], pq[:, 0:gs, :, 1],
                        op=ALU.subtract)
                    exf = sp_.tile([128, GS, H], f32, tag="exf")
                    nc.scalar.activation(exf[:, 0:gs, :], esc[:, 0:gs, :], AF.Exp,
                                         scale=1.0 / 16.0)
                    exb = sp_.tile([128, GS, H], bf16, tag="exb")
                    nc.vector.tensor_copy(exb[:, 0:gs, :], exf[:, 0:gs, :])
                    # round the numerator scalar through the SAME bf16 values
                    # the denominator matmul uses, so rounding cancels in the
                    # softmax ratio (ts scalars must be f32)
                    exf2 = sp_.tile([128, GS, H], f32, tag="exf2")
                    nc.vector.tensor_copy(exf2[:, 0:gs, :], exb[:, 0:gs, :])
                    y = wp.tile([128, GS, cout], bf16, tag="y")
                    for si in range(gs):
                        for h in range(H):
                            nc.vector.tensor_scalar_mul(
                                y[:, si, h * Cc:(h + 1) * Cc],
                                xls[:, si, h * Cc:(h + 1) * Cc],
                                exf2[:, si, h:h + 1])
                    for si in range(gs):
                        sg = g0 + si
                        b = int(blk_of_slot[sg])
                        first = sg == off[b]
                        last = sg == off[b + 1] - 1
                        if first:
                            numer_ps = epsum.tile([128, cout], f32, tag="nps")
                            denom_ps = epsum.tile([128, H], f32, tag="dps")
                        lhsT = s0_sb[:, sg * 128:(sg + 1) * 128]
                        for n in range(cout // 512):
                            sl = slice(n * 512, (n + 1) * 512)
                            nc.tensor.matmul(numer_ps[:, sl], lhsT, y[:, si, sl],
                                             start=first, stop=last)
                        nc.tensor.matmul(denom_ps[:], lhsT, exb[:, si, :],
                                         start=first, stop=last)
                        if last:
                            dn = sp_.tile([128, H], f32, tag="dn")
                            rec = sp_.tile([128, H], f32, tag="rec")
                            c1 = sp_.tile([128, H], f32, tag="c1")
                            for h in range(H):
                                nc.vector.tensor_add(
                                    dn[:, h:h + 1], denom_ps[:, h:h + 1],
                                    dummy[:, b:b + 1])
                            nc.vector.reciprocal(rec[:], dn[:])
                            for h in range(H):
                                nc.vector.tensor_mul(
                                    c1[:, h:h + 1], rec[:, h:h + 1],
                                    invdeg[:, b:b + 1])
                            for h in range(H):
                                nc.vector.tensor_scalar_mul(
                                    agg_full[:, b, h * Cc:(h + 1) * Cc],
                                    numer_ps[:, h * Cc:(h + 1) * Cc],
                                    c1[:, h:h + 1])

            # ================= transpose + BN =================
            if 4 * li + 3 >= KSTAGES:
                aggp.release()
                break
            with ExitStack() as lt:
                tps = lt.enter_context(
                    tc.tile_pool(name=f"tp{li}", bufs=4, space="PSUM"))
                tsp = lt.enter_context(tc.tile_pool(name=f"ts{li}", bufs=3))
                raws = lt.enter_context(tc.tile_pool(name=f"rw{li}", bufs=1))
                raw = ([raws.tile([128, ROWS], f32, tag=f"r{c}", name=f"raw{li}_{c}") for c in range(nch_out)] if li < 3 else None)
                for c in range(nch_out):
                    for b in range(8):
                        pt = tps.tile([128, 128], f32, tag="tp")
                        nc.tensor.transpose(
                            pt[:], agg_full[:, b, c * 128:(c + 1) * 128], eye[:])
                        if li < 3:
                            nc.scalar.activation(
                                raw[c][:, b * 128:(b + 1) * 128], pt[:], AF.Copy)
                        else:
                            nc.scalar.activation(
                                hT_next[c][:, b * 128:(b + 1) * 128], pt[:],
                                AF.Relu, scale=sc4[:, c:c + 1],
                                bias=b4p[:, c:c + 1])

                if li < 3:
                    stat = tsp.tile([128, 2 * nch_out], f32, tag="stat")
                    for c in range(nch_out):
                        nc.vector.reduce_sum(stat[:, c:c + 1], raw[c][:, 0:NPC],
                                             axis=mybir.AxisListType.X)
                        sq = tsp.tile([128, NPC], f32, tag="sq")
                        nc.scalar.activation(
                            sq[:], raw[c][:, 0:NPC], AF.Square,
                            accum_out=stat[:, nch_out + c:nch_out + c + 1])
                    st_in = dram.tile([128, 2 * nch_out], f32, tag=f"sti{li}")
                    st_out = dram.tile([128, 2 * nch_out], f32, tag=f"sto{li}")
                    nc.sync.dma_start(st_in[:], stat[:])
                    nc.gpsimd.collective_compute(
                        "AllReduce", ALU.add,
                        replica_groups=[list(range(NC))],
                        ins=[st_in[:].opt()], outs=[st_out[:].opt()])
                    gstat = tsp.tile([128, 2 * nch_out], f32, tag="gstat")
                    nc.sync.dma_start(gstat[:], st_out[:])
                    mean = tsp.tile([128, nch_out], f32, tag="mean")
                    nc.scalar.mul(mean[:], gstat[:, 0:nch_out], 1.0 / N)
                    msq = tsp.tile([128, nch_out], f32, tag="msq")
                    nc.scalar.mul(msq[:], gstat[:, nch_out:2 * nch_out], 1.0 / N)
                    m2 = tsp.tile([128, nch_out], f32, tag="m2")
                    nc.vector.tensor_mul(m2[:], mean[:], mean[:])
                    var = tsp.tile([128, nch_out], f32, tag="var")
                    nc.vector.tensor_tensor(var[:], msq[:], m2[:], op=ALU.subtract)
                    g_t, b_t, e_t = bnv[li]
                    veps = tsp.tile([128, nch_out], f32, tag="veps")
                    nc.vector.tensor_add(veps[:], var[:], e_t[:])
                    sd = tsp.tile([128, nch_out], f32, tag="sd")
                    nc.scalar.activation(sd[:], veps[:], AF.Sqrt)
                    isd = tsp.tile([128, nch_out], f32, tag="isd")
                    nc.vector.reciprocal(isd[:], sd[:])
                    sc = tsp.tile([128, nch_out], f32, tag="sc")
                    nc.vector.tensor_mul(sc[:], isd[:], g_t[:])
                    msc = tsp.tile([128, nch_out], f32, tag="msc")
                    nc.vector.tensor_mul(msc[:], mean[:], sc[:])
                    bi = tsp.tile([128, nch_out], f32, tag="bi")
                    nc.vector.tensor_tensor(bi[:], b_t[:], msc[:], op=ALU.subtract)
                    for c in range(nch_out):
                        nc.scalar.activation(
                            hT_next[c][:], raw[c][:], AF.Relu,
                            scale=sc[:, c:c + 1], bias=bi[:, c:c + 1])
            aggp.release()
            hT = hT_next

        # ================= head =================
        # out[0, n] = sum_c wh[c] * h4T[c, n]; stationary = wh chunk [128, 2]
        # (second column zero to satisfy fp32r even-free-dim), moving = h4T.
        if 16 >= KSTAGES:
            for p in sorted(hT_pool, reverse=True):
                try:
                    hT_pool[p].release()
                except Exception:
                    pass
            with tc.tile_pool(name="zt", bufs=1) as ztp:
                zt = ztp.tile([1, ROWS], f32)
                nc.gpsimd.memset(zt[:], 0.0)
                nc.sync.dma_start(pred_d[:], zt[:])
        else:
          with ExitStack() as lh:
              hps = lh.enter_context(tc.tile_pool(name="hps", bufs=2, space="PSUM"))
              hsb = lh.enter_context(tc.tile_pool(name="hsb", bufs=1))
              wh_sb = cv("whp")
              ones2 = hsb.tile([128, 2], f32)
              nc.gpsimd.memset(ones2[:], 1.0)
              # t[p, n] = sum_c wh[c*128+p] * h4T[c*128+p, n]  (per-partition)
              acc = hsb.tile([128, ROWS], f32)
              tmp = hsb.tile([128, ROWS], f32)
              nc.vector.tensor_scalar_mul(acc[:], hT[0][:].bitcast(f32),
                                          wh_sb[:, 0:1])
              for c in range(1, 4):
                  nc.vector.tensor_scalar_mul(tmp[:], hT[c][:].bitcast(f32),
                                              wh_sb[:, 2 * c:2 * c + 1])
                  nc.vector.tensor_add(acc[:], acc[:], tmp[:])
              pred_sb = hsb.tile([1, ROWS], f32)
              for n in range(2):
                  nsl = slice(n * 512, (n + 1) * 512)
                  pp = hps.tile([2, 512], f32, tag="pp")
                  nc.tensor.matmul(pp[:], ones2[:], acc[:, nsl],
                                   start=True, stop=True)
                  nc.scalar.activation(pred_sb[:, nsl], pp[0:1, :], AF.Sigmoid,
                                       bias=float(bh_val))
              nc.sync.dma_start(pred_d[:], pred_sb[:])
          hT_pool[4].release()

    nc.compile()
    _PROGRAM_CACHE[key] = (nc, SLOTS)
    return nc, SLOTS


def _host_prep(inputs):
    x = np.asarray(inputs["x"], np.float32)
    m = x.mean(0)
    v = x.var(0)
    h0 = ((x - m) / np.sqrt(v + 1e-5) * np.asarray(inputs["bn0_g"])
          + np.asarray(inputs["bn0_b"])).astype(np.float32)
    G = build_structs(np.asarray(inputs["edge_index"]))
    W = prep_weights(inputs)
    return h0, G, W


def make_in_maps(h0, G, W):
    SLOTS = G["SLOTS"]
    # replicated weight payloads, sharded by rank
    w0l_pad = np.zeros((F_PAD, 1024), BF)
    w0l_pad[:F_IN] = W["wl0"]
    w0r_pad = np.zeros((F_PAD, 1024), BF)
    w0r_pad[:F_IN] = W["wr0"]
    # merged f32 shard rows per rank: w1l(128) w1r(128) w2l(64) w2r(64)
    # w3l(64) w3r(64)
    wfs = np.empty((NC, 512, 512), np.float32)
    for c in range(NC):
        wfs[c, 0:128] = W["wl1"][c * 128:(c + 1) * 128]
        wfs[c, 128:256] = W["wr1"][c * 128:(c + 1) * 128]
        wfs[c, 256:320] = W["wl2"][c * 64:(c + 1) * 64]
        wfs[c, 320:384] = W["wr2"][c * 64:(c + 1) * 64]
        wfs[c, 384:448] = W["wl3"][c * 64:(c + 1) * 64]
        wfs[c, 448:512] = W["wr3"][c * 64:(c + 1) * 64]
    whp = np.ascontiguousarray(np.stack(
        [W["wh"].reshape(4, 128).T, np.zeros((128, 4), np.float32)],
        axis=2).reshape(128, 8))
    SH = F_PAD // NC
    in_maps = []
    for c in range(NC):
        h0p = np.ascontiguousarray(h0[c * NPC:(c + 1) * NPC].T).astype(BF)
        invdeg = np.zeros(ROWS, np.float32)
        invdeg[:NPC] = 1.0 / G["deg"][c * NPC:(c + 1) * NPC]
        dummy = np.zeros(ROWS, np.float32)
        dummy[NPC:] = 1.0
        cst_parts = [G["dst_adj"][c], whp, _pack_pp(invdeg), _pack_pp(dummy),
                     _pack_pp(W["scale4"]), _pack_pp(W["bias4"])]
        for li in (0, 1, 2):
            g, b, e = W[f"bn{li}"]
            cst_parts += [_pack_pp(g), _pack_pp(b), _pack_pp(e)]
        cst = np.concatenate(cst_parts, axis=1).astype(np.float32)
        b16 = np.concatenate([
            h0p.ravel(),
            w0l_pad[c * SH:(c + 1) * SH].ravel(),
            w0r_pad[c * SH:(c + 1) * SH].ravel(),
            _wrap_idx(G["src_pos"][c], SLOTS).ravel().view(BF),
            _wrap_idx(G["dst_pos"][c], SLOTS).ravel().view(BF),
        ]).reshape(1, -1)
        b32 = np.concatenate([wfs[c].ravel(), cst.ravel()]).reshape(1, -1)
        in_maps.append({"b16": b16, "b32": b32})
    return in_maps


_LAST_RESULTS = {}


def kernel(**inputs):
    h0, G, W = _host_prep(inputs)
    nc, SLOTS = build_program(G, W["npos"], float(np.asarray(inputs["bh"])[0]))
    in_maps = make_in_maps(h0, G, W)
    res = run_bass_kernel_spmd(nc, in_maps, core_ids=list(range(NC)))
    _LAST_RESULTS["res"] = res
    pred = np.concatenate(
        [res.results[c]["pred"].reshape(-1)[:NPC] for c in range(NC)])
    ti = np.asarray(inputs["train_idx"])
    return pred[ti].astype(np.float32), np.asarray(inputs["y"])[ti]
